# revision 29
# baseline (speedup 1.0000x reference)
"""Multi-head attention (B=2, S=2048, D=1024, H=16, DK=64) on 8 Trainium2 cores.

Sharding: 8 cores x (1 batch, 4 heads) each.  Core c handles batch c//4,
heads [4*(c%4) : 4*(c%4)+4].  Each core computes its heads' slice of the
output projection (rows of Wo for its heads); the host sums the 4 partial
outputs per batch and adds the bias.

All data stays bf16: fp8 anywhere on the value or logit path measurably
breaks the 2e-2 error budget (each fp8 stage alone contributes 2.5-4e-2,
because attention outputs are zero-mean random walks, so per-element
relative error does not average down).

The kernel is a software-pipelined stream built around the ACT exp floor
(128 x ~1.1us tiles) and the PE matmul floor (~165us of bf16 cycles):
  - input DMAs are issued smallest-deadline-first and S-chunked so the
    first scores matmul fires as soon as ~1.5MB has landed
  - q/k projections are emitted in 512-column groups; only the two groups
    unit 0 needs immediately run up front, the rest drain as deadline-
    paced fill work inside the 8 attention units
  - v-projections and the output projection drain the same way
  - attn@v lags the exp stream by 8 m-chunks (deep attnT pool) so fills
    never back-pressure the scores->exp chain; the lagged tail of each
    unit (8 attn@v + normalize) is carried into slots 0-8 of the next
  - scores own the ppool ping-pong exclusively; every fill/projection
    accumulates in popool so a fill never steals the scores buffer
  - the final unit's normalize runs split in halves so the epilogue
    output projections overlap its latency chain
"""

import numpy as np
import ml_dtypes
from contextlib import ExitStack

import concourse.bass as bass
import concourse.tile as tile
from concourse import bacc, mybir
from concourse import bass_utils

B, S, D, H, DK = 2, 2048, 1024, 16, 64
N_CORES = 8
HPC = 4            # heads per core
PAIRS = HPC // 2   # head pairs per core
KC = D // 128      # contraction chunks over D
MC = S // 128      # m (key) chunks
F32 = mybir.dt.float32
BF16 = mybir.dt.bfloat16
BF16_NP = ml_dtypes.bfloat16
AV_LAG = 8         # attn@v runs this many mc slots behind the exp stream

_COMPILED = {}


def _emit(tc, qT, kT, vT, wq, wk, wv, wo, out_dram):
    nc = tc.nc
    AFT = mybir.ActivationFunctionType
    qTa, kTa, vTa = qT.ap(), kT.ap(), vT.ap()
    wqa, wka, wva, woa = wq.ap(), wk.ap(), wv.ap(), wo.ap()
    outa = out_dram.ap()

    with ExitStack() as ctx:
        big = ctx.enter_context(tc.tile_pool(name="big", bufs=1))
        att = ctx.enter_context(tc.tile_pool(name="att", bufs=AV_LAG + 5))
        dance = ctx.enter_context(tc.tile_pool(name="dance", bufs=1))
        ostage = ctx.enter_context(tc.tile_pool(name="ostage", bufs=4))
        ppool = ctx.enter_context(tc.tile_pool(name="psum", bufs=2, space="PSUM"))
        popool = ctx.enter_context(tc.tile_pool(name="psum_o", bufs=2, space="PSUM"))

        # ---- warm the ACT exp table immediately ---------------------------
        warm_sb = big.tile([1, 64], BF16, tag="warm")
        nc.vector.memset(warm_sb[:], 1.0)
        nc.scalar.activation(warm_sb[:], warm_sb[:], AFT.Exp)

        # spin the PE during the initial DMA wait so the first projections
        # run at full clock (the p-state ramps with continuous busy time;
        # a cold 512-row matmul takes ~700ns vs ~217ns warm)
        junk_sb = big.tile([128, 512], BF16, tag="junk")
        nc.vector.memset(junk_sb[:], 0.0)
        def spin_pe(n):
            psj = ppool.tile([128, 512], F32, tag="pp", name="ps_warm")
            for i in range(n):
                nc.tensor.matmul(psj[:], junk_sb[:, 0:128], junk_sb[:],
                                 start=(i == 0), stop=(i == n - 1))

        spin_pe(22)

        # ---- SBUF tiles ---------------------------------------------------
        wq_sb = big.tile([128, KC, HPC * DK], BF16, tag="wq")
        wk_sb = big.tile([128, KC, HPC * DK], BF16, tag="wk")
        wv_sb = big.tile([128, KC, HPC * DK], BF16, tag="wv")
        wo_sb = big.tile([128, PAIRS, D], BF16, tag="wo")
        kT_sb = big.tile([128, KC, S], BF16, tag="kT")
        qT_sb = big.tile([128, KC, S], BF16, tag="qT")
        vT_sb = big.tile([128, KC, S], BF16, tag="vT")
        vh_sb = big.tile([128, MC, HPC, DK + 1], BF16, tag="vh")
        outT2_sb = big.tile([128, PAIRS, S], BF16, tag="o2")
        qhT_sb = [big.tile([128, S], BF16, tag=f"qhT{p}", name=f"qhT{p}")
                  for p in range(PAIRS)]
        khT_sb = [big.tile([128, S], BF16, tag=f"khT{p}", name=f"khT{p}")
                  for p in range(PAIRS)]

        # ones column for the softmax row sums
        nc.vector.memset(vh_sb[:], 1.0)

        # ---- input DMAs, ordered by consumption deadline ------------------
        kT_ap = kTa.rearrange("(c p) s -> p c s", p=128)
        qT_ap = qTa.rearrange("(c p) s -> p c s", p=128)
        vT_ap = vTa.rearrange("(c p) s -> p c s", p=128)
        nc.sync.dma_start(wk_sb[:], wka.rearrange("(c p) n -> p c n", p=128))
        nc.sync.dma_start(kT_sb[:, :, 0:512], kT_ap[:, :, 0:512])
        nc.sync.dma_start(wq_sb[:], wqa.rearrange("(c p) n -> p c n", p=128))
        nc.sync.dma_start(qT_sb[:, :, 0:512], qT_ap[:, :, 0:512])
        nc.sync.dma_start(qT_sb[:, :, 512:1024], qT_ap[:, :, 512:1024])
        nc.sync.dma_start(kT_sb[:, :, 512:1024], kT_ap[:, :, 512:1024])
        nc.sync.dma_start(kT_sb[:, :, 1024:1536], kT_ap[:, :, 1024:1536])
        nc.sync.dma_start(wv_sb[:], wva.rearrange("(c p) n -> p c n", p=128))
        nc.sync.dma_start(vT_sb[:, :, 0:512], vT_ap[:, :, 0:512])
        nc.sync.dma_start(kT_sb[:, :, 1536:2048], kT_ap[:, :, 1536:2048])
        nc.sync.dma_start(vT_sb[:, :, 512:1024], vT_ap[:, :, 512:1024])
        nc.sync.dma_start(qT_sb[:, :, 1024:2048], qT_ap[:, :, 1024:2048])
        nc.sync.dma_start(vT_sb[:, :, 1024:1536], vT_ap[:, :, 1024:1536])
        nc.sync.dma_start(vT_sb[:, :, 1536:2048], vT_ap[:, :, 1536:2048])
        nc.sync.dma_start(wo_sb[:], woa.rearrange("(c p) d -> p c d", p=128))

        # ---- emitters (all fills accumulate in popool) --------------------
        def emit_proj_qk(p, w_sb, src, dst, qchunk):
            """One 512-column projection group: dst[:, qchunk*512:+512]."""
            ps = popool.tile([128, 512], F32, tag="po", name="ps_proj")
            for kc in range(KC):
                nc.tensor.matmul(
                    ps[:],
                    w_sb[:, kc, p * 128:(p + 1) * 128],
                    src[:, kc, qchunk * 512:(qchunk + 1) * 512],
                    start=(kc == 0),
                    stop=(kc == KC - 1),
                )
            nc.vector.tensor_copy(dst[:, qchunk * 512:(qchunk + 1) * 512], ps[:])

        def emit_proj_v(mc):
            ps = popool.tile([128, HPC * DK], F32, tag="po", name="ps_v")
            for kc in range(KC):
                nc.tensor.matmul(
                    ps[:],
                    vT_sb[:, kc, mc * 128:(mc + 1) * 128],
                    wv_sb[:, kc, :],
                    start=(kc == 0),
                    stop=(kc == KC - 1),
                )
            nc.vector.tensor_copy(
                vh_sb[:, mc, :, 0:DK],
                ps[:].rearrange("p (h k) -> p h k", k=DK),
            )

        def emit_outproj(qi, stage_engine="vector"):
            po = popool.tile([128, 1024], F32, tag="po", name="po")
            for j in range(2):
                for p in range(PAIRS):
                    nc.tensor.matmul(
                        po[:, j * 512:(j + 1) * 512],
                        outT2_sb[:, p, qi * 128:(qi + 1) * 128],
                        wo_sb[:, p, j * 512:(j + 1) * 512],
                        start=(p == 0),
                        stop=(p == PAIRS - 1),
                    )
            so = ostage.tile([128, 1024], BF16, tag="so", name="so")
            if stage_engine == "scalar":
                nc.scalar.activation(so[:], po[:], AFT.Copy)
            else:
                nc.vector.tensor_copy(so[:], po[:])
            nc.sync.dma_start(outa[qi * 128:(qi + 1) * 128, :], so[:])

        # ---- upfront PE work: only what unit 0 needs to start -------------
        # scores(mc 0-3) read khT q-chunk 0 and BOTH qhT chunks of qc 0;
        # khT q1-q3 drain as the first fills inside unit 0.
        emit_proj_qk(0, wk_sb, kT_sb, khT_sb[0], 0)
        spin_pe(8)    # keep the clock up while qT chunk 0 lands
        emit_proj_qk(0, wq_sb, qT_sb, qhT_sb[0], 0)
        spin_pe(3)
        emit_proj_qk(0, wq_sb, qT_sb, qhT_sb[0], 1)

        def proj_halves(p, w_sb, src, dst, qchunk):
            """A projection group as two 4-kc fill bursts (half the ACT-gap
            a whole group would punch into the exp stream)."""
            state = {}
            def half(i, state=state):
                if i == 0:
                    state["ps"] = popool.tile([128, 512], F32, tag="po",
                                              name="ps_fh")
                ps = state["ps"]
                for kc in range(4 * i, 4 * i + 4):
                    nc.tensor.matmul(
                        ps[:],
                        w_sb[:, kc, p * 128:(p + 1) * 128],
                        src[:, kc, qchunk * 512:(qchunk + 1) * 512],
                        start=(kc == 0),
                        stop=(kc == KC - 1),
                    )
                if i == 1:
                    nc.vector.tensor_copy(
                        dst[:, qchunk * 512:(qchunk + 1) * 512], ps[:])
            return [lambda i=i: half(i) for i in range(2)]

        # fill order by deadline (units are pair-major); unit-0's own khT
        # groups stay whole (deadline-tight), the rest split into halves
        fill_queue = (
            [lambda q=q: emit_proj_qk(0, wk_sb, kT_sb, khT_sb[0], q)
             for q in (1, 2, 3)]                                 # u0 in-unit
            + proj_halves(0, wq_sb, qT_sb, qhT_sb[0], 2)         # u2 slot 0
            + proj_halves(0, wq_sb, qT_sb, qhT_sb[0], 3)
            + proj_halves(1, wq_sb, qT_sb, qhT_sb[1], 0)         # u4 slot 0
            + proj_halves(1, wq_sb, qT_sb, qhT_sb[1], 1)
            + proj_halves(1, wk_sb, kT_sb, khT_sb[1], 0)         # u4
            + proj_halves(1, wk_sb, kT_sb, khT_sb[1], 1)
            + proj_halves(1, wk_sb, kT_sb, khT_sb[1], 2)
            + proj_halves(1, wk_sb, kT_sb, khT_sb[1], 3)
            + proj_halves(1, wq_sb, qT_sb, qhT_sb[1], 2)         # u6 slot 0
            + proj_halves(1, wq_sb, qT_sb, qhT_sb[1], 3)
        )
        # per-unit fill slots; in units with a carried tail the popool slot
        # only frees after the carried dance (slot 8), so fills go at >=9
        FILL_SLOTS = {
            0: (1, 5, 8),
            1: (9, 11, 13, 15), 2: (9, 10, 11, 12, 13, 14),
            3: (9, 10, 11, 12, 13, 14),
            4: (9, 11, 13, 15), 5: (),
            6: (9, 11, 13, 15), 7: (),
        }

        # ---- attention units ----------------------------------------------
        def emit_av(st, mc):
            h = 2 * st["p"] + st["hh"]
            at = st["at"].pop(mc)
            for j in range(2):
                nc.tensor.matmul(
                    st["pout"][:, j * 512:(j + 1) * 512],
                    vh_sb[:, mc, h, :],
                    at[:, j * 512:(j + 1) * 512],
                    start=(mc == 0),
                    stop=(mc == MC - 1),
                )

        def emit_dance(st, half=None):
            p, hh, qc = st["p"], st["hh"], st["qc"]
            pout = st["pout"]
            if half is None:
                lo, w = 0, 1024
            else:
                lo, w = half * 512, 512
            sl = slice(lo, lo + w)
            sums = dance.tile([1, w], F32, tag="sums", name="sums")
            nc.vector.tensor_copy(sums[:], pout[64:65, sl])
            rcp32 = dance.tile([1, w], F32, tag="rcp32", name="rcp32")
            nc.vector.reciprocal_approx_fast(rcp32[:], sums[:])
            rcpb = dance.tile([64, w], F32, tag="rcpb", name="rcpb")
            nc.gpsimd.partition_broadcast(rcpb[:], rcp32[:])
            nc.vector.tensor_tensor(
                outT2_sb[hh * 64:hh * 64 + 64, p,
                         qc * 1024 + lo: qc * 1024 + lo + w],
                pout[0:64, sl],
                rcpb[:],
                mybir.AluOpType.mult,
            )

        def attention_unit(p, hh, qc, fills, carry, vfills, unit_idx):
            st = {
                "p": p, "hh": hh, "qc": qc, "at": {},
                "pout": popool.tile([65, 1024], F32, tag="po", name="pout"),
            }
            fill_slots = FILL_SLOTS[unit_idx]
            for mc in range(MC):
                # v-projections: all 16 drain inside unit 0 (unit 1 would
                # need a third popool slot against the carried pout);
                # slot 8 is left free for the khT q3 fill burst
                if vfills and unit_idx == 0 and mc >= 6 and mc != 8:
                    vfills.pop(0)()
                    if mc >= 9 and vfills:
                        vfills.pop(0)()
                # carried tail of the previous unit: 8 attn@v + dance,
                # drained over slots 0-8 so pout(prev) frees by slot 8
                if carry:
                    carry.pop(0)()
                at = att.tile([128, 1024], BF16, tag="attnT", name="at")
                st["at"][mc] = at
                ps = ppool.tile([128, 1024], F32, tag="pp", name="ps_sc")
                for j in range(2):
                    nc.tensor.matmul(
                        ps[:, j * 512:(j + 1) * 512],
                        khT_sb[p][hh * 64:hh * 64 + 64, mc * 128:(mc + 1) * 128],
                        qhT_sb[p][hh * 64:hh * 64 + 64,
                                  qc * 1024 + j * 512: qc * 1024 + (j + 1) * 512],
                        start=True,
                        stop=True,
                    )
                nc.scalar.activation(at[:], ps[:], AFT.Exp, scale=0.125)
                if mc >= AV_LAG:
                    emit_av(st, mc - AV_LAG)
                if fills and mc in fill_slots:
                    fills.pop(0)()
            tail = [lambda mc=mc: emit_av(st, mc)
                    for mc in range(MC - AV_LAG, MC)]
            tail.append(lambda: emit_dance(st))
            return tail, st

        units = [(0, 0, 0), (0, 1, 0), (0, 0, 1), (0, 1, 1),
                 (1, 0, 0), (1, 1, 0), (1, 0, 1), (1, 1, 1)]
        vfills = [(lambda mc=mc: emit_proj_v(mc)) for mc in range(MC)]
        carry = []
        st_last = None
        for u, (p, hh, qc) in enumerate(units):
            carry, st_last = attention_unit(p, hh, qc, fill_queue, carry,
                                            vfills, u)
            if u == 5:
                # qi 0-3 pop inside unit 6 (exp stream live: DVE staging
                # only); qi 4-7 drain in the epilogue where ACT is free
                for qi in range(0, 8):
                    eng = "scalar" if qi >= 4 and qi % 2 == 0 else "vector"
                    fill_queue.append(
                        lambda qi=qi, eng=eng: emit_outproj(qi, stage_engine=eng))
        # ---- epilogue: drain u7's tail; split dance so outproj overlaps ---
        for f in carry[:-1]:
            f()
        while fill_queue:
            fill_queue.pop(0)()
        emit_dance(st_last, half=0)
        for qi in (8, 9, 10, 11):
            emit_outproj(qi, stage_engine="scalar" if qi % 2 == 0 else "vector")
        emit_dance(st_last, half=1)
        for qi in (12, 13, 14, 15):
            emit_outproj(qi, stage_engine="scalar" if qi % 2 == 0 else "vector")


def build_program():
    nc = bacc.Bacc(
        "TRN2",
        target_bir_lowering=False,
        debug=False,
        enable_asserts=False,
        num_devices=N_CORES,
    )
    qT = nc.dram_tensor("qT", [D, S], BF16, kind="ExternalInput")
    kT = nc.dram_tensor("kT", [D, S], BF16, kind="ExternalInput")
    vT = nc.dram_tensor("vT", [D, S], BF16, kind="ExternalInput")
    wq = nc.dram_tensor("wq", [D, HPC * DK], BF16, kind="ExternalInput")
    wk = nc.dram_tensor("wk", [D, HPC * DK], BF16, kind="ExternalInput")
    wv = nc.dram_tensor("wv", [D, HPC * DK], BF16, kind="ExternalInput")
    wo = nc.dram_tensor("wo", [HPC * DK, D], BF16, kind="ExternalInput")
    out = nc.dram_tensor("out", [S, D], BF16, kind="ExternalOutput")
    with tile.TileContext(nc) as tc:
        _emit(tc, qT, kT, vT, wq, wk, wv, wo, out)
    nc.compile()
    return nc


def _get_program():
    if "nc" not in _COMPILED:
        _COMPILED["nc"] = build_program()
    return _COMPILED["nc"]


def make_in_maps(q, k, v, Wq, Wk, Wv, Wo):
    """Shard FULL fp32 inputs into per-core bf16 input maps."""
    q, k, v = (np.asarray(x, np.float32) for x in (q, k, v))
    Wq, Wk, Wv, Wo = (np.asarray(x, np.float32) for x in (Wq, Wk, Wv, Wo))
    qT = [np.ascontiguousarray(q[b].T).astype(BF16_NP) for b in range(B)]
    kT = [np.ascontiguousarray(k[b].T).astype(BF16_NP) for b in range(B)]
    vT = [np.ascontiguousarray(v[b].T).astype(BF16_NP) for b in range(B)]
    in_maps = []
    for c in range(N_CORES):
        b, g = divmod(c, N_CORES // B)
        heads = range(HPC * g, HPC * g + HPC)
        wq_c = np.concatenate([Wq[h] for h in heads], axis=1).astype(BF16_NP)
        wk_c = np.concatenate([Wk[h] for h in heads], axis=1).astype(BF16_NP)
        wv_c = np.concatenate([Wv[h] for h in heads], axis=1).astype(BF16_NP)
        wo_c = np.concatenate(
            [Wo[h * DK:(h + 1) * DK] for h in heads], axis=0
        ).astype(BF16_NP)
        in_maps.append({
            "qT": qT[b], "kT": kT[b], "vT": vT[b],
            "wq": np.ascontiguousarray(wq_c),
            "wk": np.ascontiguousarray(wk_c),
            "wv": np.ascontiguousarray(wv_c),
            "wo": np.ascontiguousarray(wo_c),
        })
    return in_maps


def run_on_hw(in_maps, trace=False):
    nc = _get_program()
    return bass_utils.run_bass_kernel_spmd(
        nc, in_maps, list(range(N_CORES)), trace=trace
    )


def kernel(q, k, v, Wq, Wk, Wv, Wo, bo):
    in_maps = make_in_maps(q, k, v, Wq, Wk, Wv, Wo)
    res = run_on_hw(in_maps)
    bo = np.asarray(bo, np.float32)
    parts = [r["out"].astype(np.float32) for r in res.results]
    out = np.empty((B, S, D), np.float32)
    per_b = N_CORES // B
    for b in range(B):
        out[b] = np.sum(parts[b * per_b:(b + 1) * per_b], axis=0) + bo
    return out
